# revision 5
# baseline (speedup 1.0000x reference)
"""CRF (conv features -> emissions -> Viterbi decode) Trainium2 kernel.

Contract: kernel(**inputs) takes the FULL inputs from setup_inputs() and
returns (preds [B,T] int32, score [B] float32) exactly like reference().

Strategy:
  - The 5x5 'same' conv on the (16,8) image is linear in the 128-dim feature
    vector, so it folds (on host, in f64) into an effective emission matrix
    Weff [26,128] and bias beff [26]:  emis = X @ Weff.T + beff.
  - Data-parallel over the word batch B across 8 NeuronCores (2048 words per
    core, 16 partition-blocks of 128 words).
  - Per core, per 128-word block:
      forward t=1..13:  scores[w,(n,p)] = alpha[w,p] + Trans[p,n] (+beff[n])
        computed on PE as [alpha^T;1].T @ M27 (selector-matrix trick, each
        n-group padded to 32 cols so matmul outputs stay PSUM-bank aligned),
        then one DVE grouped max-reduce [128,26g,26] -> [128,26], then
        alpha_t = mx + emis_t (emis via its own PE matmul).
      backward: recompute backpointers along the surviving path only:
        cand = alpha_{t-1} + Trans[:,lbl_t] via onehot @ Trans^T on PE,
        argmax with exact first-index tie-break via the (26-p) key trick.
"""

import os
import sys

import numpy as np

sys.path.insert(0, "/opt/trn_rl_repo")

import concourse.bass as bass  # noqa: E402
import concourse.bacc as bacc  # noqa: E402
import concourse.mybir as mybir  # noqa: E402
import concourse.tile as tile  # noqa: E402
from concourse.bass_utils import run_bass_kernel_spmd  # noqa: E402

F32 = mybir.dt.float32
I32 = mybir.dt.int32

B, T, H, W, L, E = 16384, 14, 16, 8, 26, 128
NCORES = 8
BC = B // NCORES  # 2048 words per core
P = 128  # partition block (words)
NBLK = BC // P  # 16 blocks per core
PAD = 32  # per-n group stride in the scores layout
NS = L * PAD  # 832 score columns (padded)
AX = mybir.AxisListType.X
OP = mybir.AluOpType


def _ap3(t, steps_counts):
    """Build an AP over tile t's tensor with explicit [step, count] dims."""
    a = t[:] if not isinstance(t, bass.AP) else t
    return bass.AP(a.tensor, a.offset, steps_counts)


def build_nc(nblk=NBLK):
    nc = bacc.Bacc("TRN2", target_bir_lowering=False, debug=False)
    bc = nblk * P

    xt_d = nc.dram_tensor("xt", [T, E, bc], F32, kind="ExternalInput").ap()
    m27_d = nc.dram_tensor("m27", [27, NS], F32, kind="ExternalInput").ap()
    wefft_d = nc.dram_tensor("wefft", [E, L], F32, kind="ExternalInput").ap()
    beff_d = nc.dram_tensor("beff", [1, L], F32, kind="ExternalInput").ap()
    key26_d = nc.dram_tensor("key26", [1, L], F32, kind="ExternalInput").ap()
    transt_d = nc.dram_tensor("transt", [L, L], F32, kind="ExternalInput").ap()
    ident_d = nc.dram_tensor("ident", [P, P], F32, kind="ExternalInput").ap()
    preds_d = nc.dram_tensor("preds", [bc, T], I32, kind="ExternalOutput").ap()
    score_d = nc.dram_tensor("score", [bc], F32, kind="ExternalOutput").ap()

    with tile.TileContext(nc) as tc:
        _body(tc, nblk, xt_d, m27_d, wefft_d, beff_d, key26_d, transt_d,
              ident_d, preds_d, score_d)
    nc.compile()
    return nc


def _body(tc, nblk, xt_d, m27_d, wefft_d, beff_d, key26_d, transt_d, ident_d,
          preds_d, score_d):
    nc = tc.nc
    from contextlib import ExitStack

    with ExitStack() as ctx:
        const = ctx.enter_context(tc.tile_pool(name="const", bufs=1))
        state = ctx.enter_context(tc.tile_pool(name="state", bufs=1))

        # ---- constants in SBUF ----
        m27 = const.tile([27, NS], F32)
        nc.sync.dma_start(m27, m27_d)
        wefft = const.tile([E, L], F32)
        nc.sync.dma_start(wefft, wefft_d)
        beff = const.tile([1, L], F32)
        nc.sync.dma_start(beff, beff_d)
        transt = const.tile([L, L], F32)
        nc.sync.dma_start(transt, transt_d)
        ident = const.tile([P, P], F32)
        nc.sync.dma_start(ident, ident_d)
        key26 = const.tile([P, L], F32)
        kb = key26_d
        nc.sync.dma_start(key26, bass.AP(kb.tensor, kb.offset,
                                         [[0, P], [1, L]]))
        ones1 = const.tile([1, P], F32)
        nc.vector.memset(ones1, 1.0)

        # ---- persistent per-core state ----
        alphas = state.tile([P, nblk, T, L], F32)   # all forward alphas
        alphaT = state.tile([27, nblk, P], F32)     # [alpha^T ; ones] per blk
        # row 26 must be 1.0; rows 0..25 are overwritten by per-step copies
        nc.vector.memset(alphaT[:, :, :], 1.0)
        bpv = state.tile([P, nblk, T], F32)         # key-space labels 26-p
        scv = state.tile([P, nblk], F32)            # viterbi score

        # ================= forward =================
        with ExitStack() as fctx:
            xt_pool = fctx.enter_context(tc.tile_pool(name="xt", bufs=6))
            em_sb_pool = fctx.enter_context(tc.tile_pool(name="emsb", bufs=4))
            mx_pool = fctx.enter_context(tc.tile_pool(name="mx", bufs=4))
            sc_ps = fctx.enter_context(
                tc.tile_pool(name="scps", bufs=2, space="PSUM"))
            em_ps = fctx.enter_context(
                tc.tile_pool(name="emps", bufs=2, space="PSUM"))
            tr_ps = fctx.enter_context(
                tc.tile_pool(name="trps", bufs=2, space="PSUM"))

            def load_xt(t, b):
                xt = xt_pool.tile([E, P], F32, tag="xt")
                nc.sync.dma_start(xt, xt_d[t, :, b * P:(b + 1) * P])
                return xt

            def emis_psum(xt, with_beff):
                em = em_ps.tile([P, L], F32, tag="em")
                nc.tensor.matmul(em, xt, wefft, start=True, stop=not with_beff)
                if with_beff:
                    nc.tensor.matmul(em, ones1, beff, start=False, stop=True)
                return em

            def make_alphaT(b, t):
                trp = tr_ps.tile([L, P], F32, tag="tr")
                nc.tensor.transpose(trp, alphas[:, b, t, :], ident)
                nc.scalar.copy(alphaT[0:L, b, :], trp)

            for b in range(nblk):
                xt = load_xt(0, b)
                em = emis_psum(xt, with_beff=True)
                nc.scalar.copy(alphas[:, b, 0, :], em)
                make_alphaT(b, 0)

            for t in range(1, T):
                for b in range(nblk):
                    xt = load_xt(t, b)
                    ps = sc_ps.tile([P, NS], F32, tag="sc")
                    nc.tensor.matmul(ps[:, 0:512], alphaT[:, b, :],
                                     m27[:, 0:512], start=True, stop=True)
                    nc.tensor.matmul(ps[:, 512:NS], alphaT[:, b, :],
                                     m27[:, 512:NS], start=True, stop=True)
                    em = emis_psum(xt, with_beff=False)
                    em_sb = em_sb_pool.tile([P, L], F32, tag="emsb")
                    nc.scalar.copy(em_sb, em)
                    # grouped max over p: view ps as [P, 26(n) x stride32, 26(p)]
                    ps_g = _ap3(ps, [ps[:].ap[0], [PAD, L], [1, L]])
                    mx = mx_pool.tile([P, L], F32, tag="mx")
                    nc.vector.tensor_reduce(mx, ps_g, axis=AX, op=OP.max)
                    nc.vector.tensor_tensor(alphas[:, b, t, :], mx, em_sb,
                                            op=OP.add)
                    if t < T - 1:
                        make_alphaT(b, t)

        # ================= backward =================
        with ExitStack() as bctx:
            tmp = bctx.enter_context(tc.tile_pool(name="btmp", bufs=3))
            oh_pool = bctx.enter_context(tc.tile_pool(name="oh", bufs=3))
            ohT_pool = bctx.enter_context(tc.tile_pool(name="ohT", bufs=6))
            cd_ps = bctx.enter_context(
                tc.tile_pool(name="cdps", bufs=2, space="PSUM"))
            tr2_ps = bctx.enter_context(
                tc.tile_pool(name="tr2ps", bufs=6, space="PSUM"))

            nw = 2  # waves for PE/DVE overlap
            wsz = nblk // nw if nblk >= nw else nblk
            waves = [(w * wsz, min(nblk, (w + 1) * wsz))
                     for w in range((nblk + wsz - 1) // wsz)]

            def bcast_b(ap2, n_inner):
                """[P, nb] AP -> [P, nb, n_inner] with 0-stride inner dim."""
                return bass.AP(ap2.tensor, ap2.offset,
                               [ap2.ap[0], ap2.ap[1], [0, n_inner]])

            def key_bcast(nb):
                a = key26[:]
                return bass.AP(a.tensor, a.offset, [a.ap[0], [0, nb], [1, L]])

            def argmax_batch(cand_ap, out_lbl_ap, nb, mx_src_psum=None):
                """cand [P, nb, L] -> key-space argmax into out_lbl [P, nb].

                Returns the onehot tile [P, nb, L] for the next step."""
                mxv = tmp.tile([P, nb], F32, tag="mxv")
                nc.vector.tensor_reduce(mxv, cand_ap, axis=AX, op=OP.max)
                eq = tmp.tile([P, nb, L], F32, tag="eq")
                nc.vector.tensor_tensor(eq, cand_ap, bcast_b(mxv[:], L),
                                        op=OP.is_ge)
                kv = tmp.tile([P, nb, L], F32, tag="kv")
                nc.vector.tensor_tensor(kv, eq, key_bcast(nb), op=OP.mult)
                nc.vector.tensor_reduce(out_lbl_ap, kv, axis=AX, op=OP.max)
                oh = oh_pool.tile([P, nb, L], F32, tag="oh")
                nc.vector.tensor_tensor(oh, key_bcast(nb),
                                        bcast_b(out_lbl_ap, L),
                                        op=OP.is_equal)
                return oh

            # t = T-1: score + last label, per wave
            ohs = {}
            for (w0, w1) in waves:
                nb = w1 - w0
                al = alphas[:, w0:w1, T - 1, :]
                nc.vector.tensor_reduce(scv[:, w0:w1], al, axis=AX, op=OP.max)
                cand = tmp.tile([P, nb, L], F32, tag="cd0")
                nc.vector.tensor_copy(cand, al)
                ohs[w0] = argmax_batch(cand[:], bpv[:, w0:w1, T - 1], nb)

            for t in range(T - 1, 0, -1):
                new_ohs = {}
                for (w0, w1) in waves:
                    nb = w1 - w0
                    oh = ohs[w0]
                    cd = cd_ps.tile([P, nb, L], F32, tag="cd")
                    for j in range(nb):
                        b = w0 + j
                        trp = tr2_ps.tile([L, P], F32, tag="tr2")
                        nc.tensor.transpose(trp, oh[:, j, :], ident)
                        ohT = ohT_pool.tile([L, P], F32, tag="ohT")
                        nc.scalar.copy(ohT, trp)
                        nc.tensor.matmul(cd[:, j, :], ohT, transt,
                                         start=True, stop=True)
                    cand = tmp.tile([P, nb, L], F32, tag="cand")
                    nc.vector.tensor_tensor(cand, cd,
                                            alphas[:, w0:w1, t - 1, :],
                                            op=OP.add)
                    new_ohs[w0] = argmax_batch(cand[:], bpv[:, w0:w1, t - 1],
                                               nb)
                ohs = new_ohs

            # ---- outputs ----
            predsf = tmp.tile([P, nblk, T], F32, tag="predsf")
            nc.vector.tensor_scalar(predsf, bpv, -1.0, float(L), op0=OP.mult,
                                    op1=OP.add)
            predsi = tmp.tile([P, nblk, T], I32, tag="predsi")
            nc.vector.tensor_copy(predsi, predsf)
            pd = preds_d
            nc.sync.dma_start(
                bass.AP(pd.tensor, pd.offset, [[T, P], [P * T, nblk], [1, T]]),
                predsi)
            sd = score_d
            nc.sync.dma_start(
                bass.AP(sd.tensor, sd.offset, [[1, P], [P, nblk]]), scv)


# ---------------- host side ----------------

def _fold_consts(conv_w, conv_b, params):
    cw = np.asarray(conv_w, np.float64)[0, 0]
    cb = np.float64(np.asarray(conv_b)[0])
    params = np.asarray(params)
    Wmat = params[:L * E].reshape(L, E).astype(np.float64)
    Trans = params[L * E:].reshape(L, L).astype(np.float32)

    K = np.zeros((E, E), np.float64)
    for r in range(H):
        for c in range(W):
            o = r * W + c
            for dy in range(5):
                for dx in range(5):
                    rr, cc = r + dy - 2, c + dx - 2
                    if 0 <= rr < H and 0 <= cc < W:
                        K[o, rr * W + cc] += cw[dy, dx]
    Weff = (Wmat @ K).astype(np.float32)            # [L, E]
    beff = (cb * Wmat.sum(1)).astype(np.float32)    # [L]

    # M27 [27, NS]: selector rows + (Trans[p,n] + beff[n]) row, 32-padded
    m27 = np.zeros((27, NS), np.float32)
    for n in range(L):
        for p in range(L):
            m27[p, n * PAD + p] = 1.0
            m27[26, n * PAD + p] = Trans[p, n] + beff[n]
    wefft = np.ascontiguousarray(Weff.T)            # [E, L]
    key26 = (L - np.arange(L)).astype(np.float32)[None, :]   # 26-p
    transt = np.ascontiguousarray(Trans.T)          # [n, p] = Trans[p, n]
    ident = np.eye(P, dtype=np.float32)
    return m27, wefft, beff[None, :].astype(np.float32), key26, transt, ident


_NC_CACHE = {}


def _get_nc(nblk):
    if nblk not in _NC_CACHE:
        _NC_CACHE[nblk] = build_nc(nblk)
    return _NC_CACHE[nblk]


def kernel(X, conv_w, conv_b, params):
    X = np.asarray(X, np.float32)
    m27, wefft, beff, key26, transt, ident = _fold_consts(conv_w, conv_b,
                                                          params)
    in_maps = []
    for c in range(NCORES):
        xc = X[c * BC:(c + 1) * BC]                  # [BC, T, E]
        xt = np.ascontiguousarray(xc.transpose(1, 2, 0))  # [T, E, BC]
        in_maps.append({
            "xt": xt, "m27": m27, "wefft": wefft, "beff": beff,
            "key26": key26, "transt": transt, "ident": ident,
        })
    nc = _get_nc(NBLK)
    res = run_bass_kernel_spmd(nc, in_maps, core_ids=list(range(NCORES)))
    preds = np.concatenate([r["preds"] for r in res.results], axis=0)
    score = np.concatenate([r["score"] for r in res.results], axis=0)
    return preds.astype(np.int32), score.astype(np.float32)
